# revision 5
# baseline (speedup 1.0000x reference)
"""3-layer GAT on 8 Trainium2 NeuronCores (Bass/Tile).

Sharding: 2D graph partition. Pair q = cores {2q, 2q+1} aggregates the dst
nodes of strips [q*6250,(q+1)*6250) and [25000+q*6250, 25000+(q+1)*6250);
even cores take edges with src < 25000, odd cores the rest. Node ownership:
core 2k owns rows [k*6250,(k+1)*6250), core 2k+1 owns [25000+k*6250, ...).

Per layer: each core projects its own rows (feat|el|er via an augmented
weight matrix), AllGathers a bf16 gather-table (feat_hi|el_hi|el_lo) across
its src-half quad and an fp32 er-table across its pair, then streams edges
(pre-sorted by dst, padded per 128-dst block) through: dma_gather of source
rows, edge softmax without segment-max (e-values are small), messages
accumulated per dst block by one-hot-mask matmuls into PSUM. Partial sums
are pairwise ReduceScattered, then divide/bias/ELU (head-mean on the last
layer) produce the output rows each core owns.
"""

import numpy as np
import ml_dtypes

N = 50000
E = 800000
F = 128                  # input feats and hidden width (4 heads x 32)
H = 4
D = 32
NEG = 0.2
NCORE = 8
NPC = 6250               # nodes owned per core
OWN = 6272               # 49*128, padded own rows
OWNBLK = 49
PAIR = 12544             # 98*128 dst slots per pair
NBLK = 98
HALF = 25088             # 4*OWN rows per src-half table
TROWS = 25216            # HALF + 128 (dummy row at HALF)
DUMMY = HALF
TCOLS = 256              # bf16 cols: feat_hi(128) | el_hi(4) | el_lo(4) | pad
ERCOLS = 64              # fp32 cols: er(4) | pad
CHUNK = 32               # tiles per dma_gather call
EPS = 1e-30

_cache = {}


def _preprocess(src, dst):
    src = np.asarray(src).astype(np.int64)
    dst = np.asarray(dst).astype(np.int64)
    q = np.where(dst < 25000, dst // NPC, (dst - 25000) // NPC)
    s = (src >= 25000).astype(np.int64)
    core_of = 2 * q + s
    # pair-local dst row in [0, 12544): strip A -> [0,6250), strip B -> 6272+
    rloc = np.where(dst < 25000, dst - q * NPC, OWN + (dst - 25000 - q * NPC))
    blk = rloc // 128
    slot = rloc % 128
    # table-local src index within its half
    ks = np.where(src < 25000, src // NPC, (src - 25000) // NPC)
    tloc = np.where(src < 25000, OWN * ks + src - ks * NPC,
                    OWN * ks + (src - 25000) - ks * NPC)

    # per (core, block) edge lists; uniform tiles per block across cores
    counts = np.zeros((NCORE, NBLK), np.int64)
    for c in range(NCORE):
        m = core_of == c
        counts[c] = np.bincount(blk[m], minlength=NBLK)
    T_b = np.maximum(1, np.ceil(counts.max(axis=0) / 128).astype(np.int64))
    T = int(T_b.sum())
    Tpad = ((T + 3) // 4) * 4          # pad tile count to a multiple of 4
    extra = Tpad - T
    T_b[-1] += extra
    T = Tpad

    per_core = []
    order = np.lexsort((rloc, core_of))
    srt_core = core_of[order]
    srt_rloc = rloc[order]
    srt_tloc = tloc[order]
    srt_slot = slot[order]
    srt_blk = srt_rloc // 128
    for c in range(NCORE):
        sel = srt_core == c
        cb = srt_blk[sel]
        ct = srt_tloc[sel]
        cr = srt_rloc[sel]
        csl = srt_slot[sel]
        idx_t = np.full(T * 128, DUMMY, np.int64)      # feat-table row per edge
        idx_e = np.zeros(T * 128, np.int64)            # er-table row per edge
        slots = np.zeros(T * 128, np.int64)
        pos = 0
        start = np.searchsorted(cb, np.arange(NBLK))
        end = np.searchsorted(cb, np.arange(NBLK) + 1)
        for b in range(NBLK):
            n = end[b] - start[b]
            idx_t[pos:pos + n] = ct[start[b]:end[b]]
            idx_e[pos:pos + n] = cr[start[b]:end[b]]
            slots[pos:pos + n] = csl[start[b]:end[b]]
            pos += int(T_b[b]) * 128
        per_core.append((idx_t, idx_e, slots))

    # tile -> block map
    tile_block = np.repeat(np.arange(NBLK), T_b)
    tile_first = np.zeros(T, bool)
    tile_last = np.zeros(T, bool)
    p = 0
    for b in range(NBLK):
        tile_first[p] = True
        tile_last[p + int(T_b[b]) - 1] = True
        p += int(T_b[b])

    def wrap16(a):
        # value i of each 128-group at [i%16, i//16], replicated per 16 rows
        t = a.reshape(-1, 128)                     # [T, 128]
        w = t.reshape(t.shape[0], 8, 16)           # [T, 8, 16]
        w = w.transpose(2, 0, 1).reshape(16, -1)   # [16, T*8]
        return np.tile(w, (8, 1)).astype(np.int16) # [128, T*8]

    cores = []
    for c in range(NCORE):
        idx_t, idx_e, slots = per_core[c]
        cores.append(dict(
            idxw=wrap16(idx_t),
            idx2w=wrap16(idx_e),
            slot=slots.reshape(T, 128).T.astype(ml_dtypes.bfloat16).copy(),
        ))
    return cores, tile_block, tile_first, tile_last, T


def _own_rows(c):
    k = c // 2
    if c % 2 == 0:
        return k * NPC, (k + 1) * NPC
    return 25000 + k * NPC, 25000 + (k + 1) * NPC


def _augment(W, al, ar):
    dout = W.shape[1] // H
    Wal = np.stack([W[:, h * dout:(h + 1) * dout] @ al[h] for h in range(H)], 1)
    War = np.stack([W[:, h * dout:(h + 1) * dout] @ ar[h] for h in range(H)], 1)
    return np.concatenate([W, Wal, War], 1).astype(np.float32)  # [128, 136]


def _build(tile_block, tile_first, tile_last, T, consts):
    import concourse.bass as bass
    import concourse.bacc as bacc
    import concourse.tile as tile
    from concourse import mybir
    from concourse.library_config import mlp

    f32 = mybir.dt.float32
    bf16 = mybir.dt.bfloat16
    i16 = mybir.dt.int16
    AF = mybir.ActivationFunctionType
    OP = mybir.AluOpType

    nc = bacc.Bacc(num_devices=NCORE)
    xT_in = nc.declare_dram_parameter("xT", [128, OWN], f32, isOutput=False)
    idxw_in = nc.declare_dram_parameter("idxw", [128, T * 8], i16, isOutput=False)
    idx2w_in = nc.declare_dram_parameter("idx2w", [128, T * 8], i16, isOutput=False)
    slot_in = nc.declare_dram_parameter("slot", [128, T], bf16, isOutput=False)
    y_out = nc.declare_dram_parameter("y", [NPC, D], f32, isOutput=True)

    chunks = []
    t0 = 0
    while t0 < T:
        chunks.append((t0, min(CHUNK, T - t0)))
        t0 += CHUNK

    with tile.TileContext(nc) as tc:
        with tc.tile_pool(name="persist", bufs=1) as pp, \
             tc.tile_pool(name="dram", bufs=1, space="DRAM") as dp:
            nc.gpsimd.load_library(mlp)

            # ---- persistent SBUF state ----
            idx_sb = pp.tile([128, T * 8], i16)
            nc.sync.dma_start(out=idx_sb[:], in_=idxw_in[:, :])
            idx2_sb = pp.tile([128, T * 8], i16)
            nc.sync.dma_start(out=idx2_sb[:], in_=idx2w_in[:, :])
            slot_sb = pp.tile([128, T, 1], bf16)
            nc.sync.dma_start(out=slot_sb[:, :, 0], in_=slot_in[:, :])
            hT = pp.tile([128, OWN], f32)
            nc.sync.dma_start(out=hT[:], in_=xT_in[:, :])
            hT2 = pp.tile([128, OWN], f32)

            iota_h = nc.inline_tensor(
                np.tile(np.arange(128).astype(ml_dtypes.bfloat16), (128, 1)),
                name="iota_row")
            iota_sb = pp.tile([128, 128], bf16)
            nc.sync.dma_start(out=iota_sb[:], in_=iota_h[:, :])
            ident_h = nc.inline_tensor(np.eye(128, dtype=np.float32), name="ident")
            ident_sb = pp.tile([128, 128], f32)
            nc.sync.dma_start(out=ident_sb[:], in_=ident_h[:, :])

            waug_sb = []
            brep_sb = []
            for li in range(3):
                wh = nc.inline_tensor(consts[f"Waug{li}"], name=f"waug{li}")
                wt = pp.tile([128, 136], f32, name=f"waug_sb{li}")
                nc.sync.dma_start(out=wt[:], in_=wh[:, :])
                waug_sb.append(wt)
                bh = nc.inline_tensor(consts[f"brep{li}"], name=f"brep{li}")
                bt = pp.tile([128, consts[f"brep{li}"].shape[1]], f32,
                             name=f"brep_sb{li}")
                nc.sync.dma_start(out=bt[:], in_=bh[:, :])
                brep_sb.append(bt)

            # dummy table row (feat=0, el_hi=-1e30, el_lo=0)
            dummy_h = nc.inline_tensor(consts["dummyrow"], name="dummyrow")

            # ---- DRAM scratch ----
            table = dp.tile([TROWS, TCOLS], bf16)
            er_tab = dp.tile([PAIR, ERCOLS], f32)
            ag_feat = dp.tile([OWN, TCOLS], bf16)
            ag_er = dp.tile([OWN, ERCOLS], f32)
            partial = dp.tile([PAIR, 132], f32)
            own_sum = dp.tile([OWN, 132], f32)

            nc.sync.dma_start(out=table[DUMMY:DUMMY + 1, :], in_=dummy_h[:, :])

            me = nc.my_core_id if hasattr(nc, "my_core_id") else None
            groups_pair = [[2 * k, 2 * k + 1] for k in range(4)]
            groups_quad = [[0, 2, 4, 6], [1, 3, 5, 7]]

            for li in range(3):
                src_hT = hT if li % 2 == 0 else hT2
                dst_hT = hT2 if li % 2 == 0 else hT
                last = li == 2

                # ---- projection of own rows + table assembly ----
                with tc.tile_pool(name=f"prj{li}", bufs=3) as sp, \
                     tc.tile_pool(name=f"prjps{li}", bufs=2, space="PSUM") as ps:
                    tabrow = sp.tile([128, OWNBLK, TCOLS], bf16, name=f"tabrow{li}",
                                     tag="tabrow", bufs=1)
                    errow = sp.tile([128, OWNBLK, ERCOLS], f32, name=f"errow{li}",
                                    tag="errow", bufs=1)
                    for t in range(OWNBLK):
                        pj = ps.tile([128, 136], f32, space="PSUM")
                        nc.tensor.matmul(pj[:], lhsT=src_hT[:, t * 128:(t + 1) * 128],
                                         rhs=waug_sb[li][:], start=True, stop=True)
                        # bf16 hi part: feat + el_hi
                        nc.vector.tensor_copy(tabrow[:, t, 0:132], pj[:, 0:132])
                        # el_lo = el - fp32(el_hi)
                        nc.vector.tensor_tensor(
                            out=tabrow[:, t, 132:136], in0=pj[:, 128:132],
                            in1=tabrow[:, t, 128:132], op=OP.subtract)
                        nc.scalar.activation(errow[:, t, 0:4], pj[:, 132:136], AF.Copy)
                    nc.sync.dma_start(
                        out=ag_feat[:, :].rearrange("(t p) c -> p t c", p=128),
                        in_=tabrow[:])
                    nc.sync.dma_start(
                        out=ag_er[:, :].rearrange("(t p) c -> p t c", p=128),
                        in_=errow[:])

                nc.gpsimd.collective_compute(
                    "AllGather", mybir.AluOpType.bypass,
                    replica_groups=groups_quad,
                    ins=[ag_feat[:, :]], outs=[table[0:HALF, :]])
                nc.gpsimd.collective_compute(
                    "AllGather", mybir.AluOpType.bypass,
                    replica_groups=groups_pair,
                    ins=[ag_er[:, :]], outs=[er_tab[:, :]])

                # ---- edge phase ----
                with tc.tile_pool(name=f"gt{li}", bufs=3) as gp, \
                     tc.tile_pool(name=f"ms{li}", bufs=4) as mp, \
                     tc.tile_pool(name=f"sm{li}", bufs=4) as smp, \
                     tc.tile_pool(name=f"ex{li}", bufs=4) as xp, \
                     tc.tile_pool(name=f"pb{li}", bufs=4) as pbp, \
                     tc.tile_pool(name=f"sg{li}", bufs=6, space="PSUM") as sgps:
                    seg = None
                    for (c0, clen) in chunks:
                        g = gp.tile([128, CHUNK, TCOLS], bf16, tag="g")
                        nc.gpsimd.dma_gather(
                            out_ap=g[:, 0:clen, :], in_ap=table[:, :],
                            idxs_ap=idx_sb[:, c0 * 8:(c0 + clen) * 8],
                            num_idxs=clen * 128, num_idxs_reg=clen * 128,
                            elem_size=TCOLS, single_packet=False)
                        ger = gp.tile([128, CHUNK, ERCOLS], f32, tag="ger")
                        nc.gpsimd.dma_gather(
                            out_ap=ger[:, 0:clen, :], in_ap=er_tab[:, :],
                            idxs_ap=idx2_sb[:, c0 * 8:(c0 + clen) * 8],
                            num_idxs=clen * 128, num_idxs_reg=clen * 128,
                            elem_size=ERCOLS, single_packet=False)
                        for g0 in range(0, clen, 4):
                            gl = min(4, clen - g0)
                            # masks: smat[p, t, i] = (slot[p, c0+g0+t] == i)
                            smat = smp.tile([128, 4, 128], bf16, tag="smat")
                            nc.vector.tensor_tensor(
                                out=smat[:, 0:gl, :],
                                in0=slot_sb[:, c0 + g0:c0 + g0 + gl, :]
                                    .to_broadcast([128, gl, 128]),
                                in1=iota_sb[:].rearrange("p (t i) -> p t i", t=1)
                                    .to_broadcast([128, gl, 128]),
                                op=OP.is_equal)
                            # e = el_hi + el_lo + er ; lrelu; exp
                            e4 = xp.tile([128, 4, 4], f32, tag="e4")
                            nc.vector.tensor_tensor(
                                out=e4[:, 0:gl, :], in0=g[:, g0:g0 + gl, 128:132],
                                in1=g[:, g0:g0 + gl, 132:136], op=OP.add)
                            nc.vector.tensor_tensor(
                                out=e4[:, 0:gl, :], in0=e4[:, 0:gl, :],
                                in1=ger[:, g0:g0 + gl, 0:4], op=OP.add)
                            t4 = xp.tile([128, 4, 4], f32, tag="t4")
                            nc.vector.tensor_scalar_mul(t4[:, 0:gl, :],
                                                        e4[:, 0:gl, :], NEG)
                            nc.vector.tensor_tensor(
                                out=e4[:, 0:gl, :], in0=e4[:, 0:gl, :],
                                in1=t4[:, 0:gl, :], op=OP.max)
                            ex4 = xp.tile([128, 4, 4, 1], f32, tag="ex4")
                            nc.scalar.activation(ex4[:, 0:gl, :, 0], e4[:, 0:gl, :],
                                                 AF.Exp)
                            # messages
                            m4 = mp.tile([128, 4, 132], bf16, tag="m4")
                            nc.vector.tensor_copy(m4[:, 0:gl, 128:132],
                                                  ex4[:, 0:gl, :, 0])
                            nc.vector.tensor_tensor(
                                out=m4[:, 0:gl, 0:128], in0=g[:, g0:g0 + gl, 0:128],
                                in1=ex4[:, 0:gl, :, :].to_broadcast([128, gl, 4, 32]),
                                op=OP.mult)
                            for t in range(gl):
                                gt = c0 + g0 + t
                                b = int(tile_block[gt])
                                if tile_first[gt]:
                                    seg = sgps.tile([128, 132], f32, space="PSUM",
                                                    tag="seg", name=f"seg{li}_{b}")
                                nc.tensor.matmul(
                                    seg[:], lhsT=smat[:, t, :], rhs=m4[:, t, :],
                                    start=bool(tile_first[gt]),
                                    stop=bool(tile_last[gt]))
                                if tile_last[gt]:
                                    pb = pbp.tile([128, 132], f32, tag="pb")
                                    nc.scalar.activation(pb[:], seg[:], AF.Copy)
                                    nc.sync.dma_start(
                                        out=partial[b * 128:(b + 1) * 128, :],
                                        in_=pb[:])

                nc.gpsimd.collective_compute(
                    "ReduceScatter", mybir.AluOpType.add,
                    replica_groups=groups_pair,
                    ins=[partial[:, :]], outs=[own_sum[:, :]])

                # ---- post-processing of own rows ----
                with tc.tile_pool(name=f"po{li}", bufs=4) as pop, \
                     tc.tile_pool(name=f"pops{li}", bufs=2, space="PSUM") as tps:
                    osum = pop.tile([128, OWNBLK, 132], f32, name=f"osum{li}",
                                    tag="osum", bufs=1)
                    nc.sync.dma_start(
                        out=osum[:],
                        in_=own_sum[:, :].rearrange("(t p) c -> p t c", p=128))
                    for b in range(OWNBLK):
                        rows = 128 if b < OWNBLK - 1 else NPC - 128 * (OWNBLK - 1)
                        den = pop.tile([128, 4], f32, tag="den")
                        nc.vector.tensor_scalar_max(den[:], osum[:, b, 128:132], EPS)
                        rec = pop.tile([128, 4, 1], f32, tag="rec")
                        nc.vector.reciprocal(rec[:, :, 0], den[:])
                        o = pop.tile([128, 4, 32], f32, tag="o")
                        nc.vector.tensor_tensor(
                            out=o[:], in0=osum[:, b, 0:128],
                            in1=rec[:].to_broadcast([128, 4, 32]), op=OP.mult)
                        if not last:
                            o2 = pop.tile([128, 128], f32, tag="o2")
                            nc.vector.tensor_tensor(
                                out=o2[:], in0=o[:], in1=brep_sb[li][:], op=OP.add)
                            # ELU: max(x,0) + exp(min(x,0)) - 1
                            mn = pop.tile([128, 128], f32, tag="mn")
                            nc.vector.tensor_scalar_min(mn[:], o2[:], 0.0)
                            exn = pop.tile([128, 128], f32, tag="exn")
                            nc.scalar.activation(exn[:], mn[:], AF.Exp)
                            nc.vector.tensor_scalar_max(o2[:], o2[:], 0.0)
                            nc.vector.tensor_tensor(out=o2[:], in0=o2[:],
                                                    in1=exn[:], op=OP.add)
                            nc.vector.tensor_scalar_add(o2[:], o2[:], -1.0)
                            # transpose into next layer's hT
                            tp = tps.tile([128, 128], f32, space="PSUM")
                            nc.tensor.matmul(tp[:], lhsT=o2[:], rhs=ident_sb[:],
                                             start=True, stop=True)
                            nc.scalar.activation(dst_hT[:, b * 128:(b + 1) * 128],
                                                 tp[:], AF.Copy)
                        else:
                            r1 = pop.tile([128, 32], f32, tag="r1")
                            nc.vector.tensor_tensor(out=r1[:], in0=o[:, 0, :],
                                                    in1=o[:, 1, :], op=OP.add)
                            r2 = pop.tile([128, 32], f32, tag="r2")
                            nc.vector.tensor_tensor(out=r2[:], in0=o[:, 2, :],
                                                    in1=o[:, 3, :], op=OP.add)
                            nc.vector.tensor_tensor(out=r1[:], in0=r1[:],
                                                    in1=r2[:], op=OP.add)
                            nc.vector.tensor_scalar_mul(r1[:], r1[:], 0.25)
                            nc.vector.tensor_tensor(out=r1[:], in0=r1[:],
                                                    in1=brep_sb[li][:], op=OP.add)
                            nc.sync.dma_start(
                                out=y_out[b * 128:b * 128 + rows, :],
                                in_=r1[0:rows, :])
    nc.finalize()
    return nc


def kernel(x, src, dst, W0, al0, ar0, b0, W1, al1, ar1, b1, W2, al2, ar2, b2):
    from concourse.bass_utils import run_bass_kernel_spmd

    x = np.asarray(x, dtype=np.float32)
    key = (hash(np.asarray(src).tobytes()) ^ hash(np.asarray(dst).tobytes()))
    if "pre" not in _cache or _cache.get("prekey") != key:
        _cache["pre"] = _preprocess(src, dst)
        _cache["prekey"] = key
    cores, tile_block, tile_first, tile_last, T = _cache["pre"]

    consts = {}
    for li, (W, al, ar, b) in enumerate(
            [(W0, al0, ar0, b0), (W1, al1, ar1, b1), (W2, al2, ar2, b2)]):
        consts[f"Waug{li}"] = _augment(np.asarray(W, np.float32),
                                       np.asarray(al, np.float32),
                                       np.asarray(ar, np.float32))
        b = np.asarray(b, np.float32)
        if li < 2:
            consts[f"brep{li}"] = np.tile(b.reshape(1, 128), (128, 1))
        else:
            consts[f"brep{li}"] = np.tile(b.reshape(H, D).mean(0).reshape(1, D),
                                          (128, 1))
    dummy = np.zeros((1, TCOLS), ml_dtypes.bfloat16)
    dummy[0, 128:132] = ml_dtypes.bfloat16(-1e30)
    consts["dummyrow"] = dummy

    ck = key ^ hash(consts["Waug0"].tobytes())
    if "nc" not in _cache or _cache.get("nckey") != ck:
        _cache["nc"] = _build(tile_block, tile_first, tile_last, T, consts)
        _cache["nckey"] = ck
    nc = _cache["nc"]

    in_maps = []
    for c in range(NCORE):
        lo, hi = _own_rows(c)
        xT = np.zeros((128, OWN), np.float32)
        xT[:, 0:NPC] = x[lo:hi].T
        in_maps.append(dict(xT=xT, idxw=cores[c]["idxw"],
                            idx2w=cores[c]["idx2w"],
                            slot=np.asarray(cores[c]["slot"])))
    r = run_bass_kernel_spmd(nc, in_maps, list(range(NCORE)))
    y = np.zeros((N, D), np.float32)
    for c in range(NCORE):
        lo, hi = _own_rows(c)
        y[lo:hi] = r.results[c]["y"]
    return y


# revision 6
# speedup vs baseline: 1.1305x; 1.1305x over previous
"""3-layer GAT on 8 Trainium2 NeuronCores (Bass/Tile).

Sharding: 2D graph partition. Pair q = cores {2q, 2q+1} aggregates the dst
nodes of strips [q*6250,(q+1)*6250) and [25000+q*6250, 25000+(q+1)*6250);
even cores take edges with src < 25000, odd cores the rest. Node ownership:
core 2k owns rows [k*6250,(k+1)*6250), core 2k+1 owns [25000+k*6250, ...).

Per layer: each core projects its own rows (feat|el|er via an augmented
weight matrix), AllGathers a bf16 gather-table (feat_hi|el_hi|el_lo) across
its src-half quad and an fp32 er-table across its pair, then streams edges
(pre-sorted by dst, padded per 128-dst block) through: dma_gather of source
rows, edge softmax without segment-max (e-values are small), messages
accumulated per dst block by one-hot-mask matmuls into PSUM. Partial sums
are pairwise ReduceScattered, then divide/bias/ELU (head-mean on the last
layer) produce the output rows each core owns.
"""

import numpy as np
import ml_dtypes

N = 50000
E = 800000
F = 128                  # input feats and hidden width (4 heads x 32)
H = 4
D = 32
NEG = 0.2
NCORE = 8
NPC = 6250               # nodes owned per core
OWN = 6272               # 49*128, padded own rows
OWNBLK = 49
PAIR = 12544             # 98*128 dst slots per pair
NBLK = 98
HALF = 25088             # 4*OWN rows per src-half table
TROWS = 25216            # HALF + 128 (dummy row at HALF)
DUMMY = HALF
TCOLS = 256              # bf16 cols: feat_hi(128) | el_hi(4) | el_lo(4) | pad
ERCOLS = 64              # fp32 cols: er(4) | pad
CHUNK = 32               # tiles per dma_gather call
EPS = 1e-30

_cache = {}


def _preprocess(src, dst):
    src = np.asarray(src).astype(np.int64)
    dst = np.asarray(dst).astype(np.int64)
    q = np.where(dst < 25000, dst // NPC, (dst - 25000) // NPC)
    s = (src >= 25000).astype(np.int64)
    core_of = 2 * q + s
    # pair-local dst row in [0, 12544): strip A -> [0,6250), strip B -> 6272+
    rloc = np.where(dst < 25000, dst - q * NPC, OWN + (dst - 25000 - q * NPC))
    blk = rloc // 128
    slot = rloc % 128
    # table-local src index within its half
    ks = np.where(src < 25000, src // NPC, (src - 25000) // NPC)
    tloc = np.where(src < 25000, OWN * ks + src - ks * NPC,
                    OWN * ks + (src - 25000) - ks * NPC)

    # per (core, block) edge lists; uniform tiles per block across cores
    counts = np.zeros((NCORE, NBLK), np.int64)
    for c in range(NCORE):
        m = core_of == c
        counts[c] = np.bincount(blk[m], minlength=NBLK)
    T_b = np.maximum(1, np.ceil(counts.max(axis=0) / 128).astype(np.int64))
    T = int(T_b.sum())
    Tpad = ((T + 3) // 4) * 4          # pad tile count to a multiple of 4
    extra = Tpad - T
    T_b[-1] += extra
    T = Tpad

    per_core = []
    order = np.lexsort((rloc, core_of))
    srt_core = core_of[order]
    srt_rloc = rloc[order]
    srt_tloc = tloc[order]
    srt_slot = slot[order]
    srt_blk = srt_rloc // 128
    for c in range(NCORE):
        sel = srt_core == c
        cb = srt_blk[sel]
        ct = srt_tloc[sel]
        cr = srt_rloc[sel]
        csl = srt_slot[sel]
        idx_t = np.full(T * 128, DUMMY, np.int64)      # feat-table row per edge
        idx_e = np.zeros(T * 128, np.int64)            # er-table row per edge
        slots = np.zeros(T * 128, np.int64)
        pos = 0
        start = np.searchsorted(cb, np.arange(NBLK))
        end = np.searchsorted(cb, np.arange(NBLK) + 1)
        for b in range(NBLK):
            n = end[b] - start[b]
            idx_t[pos:pos + n] = ct[start[b]:end[b]]
            idx_e[pos:pos + n] = cr[start[b]:end[b]]
            slots[pos:pos + n] = csl[start[b]:end[b]]
            pos += int(T_b[b]) * 128
        per_core.append((idx_t, idx_e, slots))

    # tile -> block map
    tile_block = np.repeat(np.arange(NBLK), T_b)
    tile_first = np.zeros(T, bool)
    tile_last = np.zeros(T, bool)
    p = 0
    for b in range(NBLK):
        tile_first[p] = True
        tile_last[p + int(T_b[b]) - 1] = True
        p += int(T_b[b])

    def wrap16(a):
        # value i of each 128-group at [i%16, i//16], replicated per 16 rows
        t = a.reshape(-1, 128)                     # [T, 128]
        w = t.reshape(t.shape[0], 8, 16)           # [T, 8, 16]
        w = w.transpose(2, 0, 1).reshape(16, -1)   # [16, T*8]
        return np.tile(w, (8, 1)).astype(np.int16) # [128, T*8]

    cores = []
    for c in range(NCORE):
        idx_t, idx_e, slots = per_core[c]
        cores.append(dict(
            idxw=wrap16(idx_t),
            idx2w=wrap16(idx_e),
            slot=slots.reshape(T, 128).T.astype(ml_dtypes.bfloat16).copy(),
        ))
    return cores, tile_block, tile_first, tile_last, T


def _own_rows(c):
    k = c // 2
    if c % 2 == 0:
        return k * NPC, (k + 1) * NPC
    return 25000 + k * NPC, 25000 + (k + 1) * NPC


def _augment(W, al, ar):
    dout = W.shape[1] // H
    Wal = np.stack([W[:, h * dout:(h + 1) * dout] @ al[h] for h in range(H)], 1)
    War = np.stack([W[:, h * dout:(h + 1) * dout] @ ar[h] for h in range(H)], 1)
    return np.concatenate([W, Wal, War], 1).astype(np.float32)  # [128, 136]


def _build(tile_block, tile_first, tile_last, T, consts):
    import concourse.bass as bass
    import concourse.bacc as bacc
    import concourse.tile as tile
    from concourse import mybir
    from concourse.library_config import mlp

    f32 = mybir.dt.float32
    bf16 = mybir.dt.bfloat16
    i16 = mybir.dt.int16
    AF = mybir.ActivationFunctionType
    OP = mybir.AluOpType

    nc = bacc.Bacc(num_devices=NCORE)
    xT_in = nc.declare_dram_parameter("xT", [128, OWN], f32, isOutput=False)
    idxw_in = nc.declare_dram_parameter("idxw", [128, T * 8], i16, isOutput=False)
    idx2w_in = nc.declare_dram_parameter("idx2w", [128, T * 8], i16, isOutput=False)
    slot_in = nc.declare_dram_parameter("slot", [128, T], bf16, isOutput=False)
    y_out = nc.declare_dram_parameter("y", [NPC, D], f32, isOutput=True)

    chunks = []
    t0 = 0
    while t0 < T:
        chunks.append((t0, min(CHUNK, T - t0)))
        t0 += CHUNK

    with tile.TileContext(nc) as tc:
        with tc.tile_pool(name="persist", bufs=1) as pp, \
             tc.tile_pool(name="dram", bufs=1, space="DRAM") as dp:
            nc.gpsimd.load_library(mlp)

            # ---- persistent SBUF state ----
            idx_sb = pp.tile([128, T * 8], i16)
            nc.sync.dma_start(out=idx_sb[:], in_=idxw_in[:, :])
            idx2_sb = pp.tile([128, T * 8], i16)
            nc.sync.dma_start(out=idx2_sb[:], in_=idx2w_in[:, :])
            slot_sb = pp.tile([128, T, 1], bf16)
            nc.sync.dma_start(out=slot_sb[:, :, 0], in_=slot_in[:, :])
            hT = pp.tile([128, OWN], f32)
            nc.sync.dma_start(out=hT[:], in_=xT_in[:, :])
            hT2 = pp.tile([128, OWN], f32)

            iota_h = nc.inline_tensor(
                np.tile(np.arange(128).astype(ml_dtypes.bfloat16), (128, 1)),
                name="iota_row")
            iota_sb = pp.tile([128, 128], bf16)
            nc.sync.dma_start(out=iota_sb[:], in_=iota_h[:, :])
            ident_h = nc.inline_tensor(np.eye(128, dtype=np.float32), name="ident")
            ident_sb = pp.tile([128, 128], f32)
            nc.sync.dma_start(out=ident_sb[:], in_=ident_h[:, :])

            waug_sb = []
            brep_sb = []
            for li in range(3):
                wh = nc.inline_tensor(consts[f"Waug{li}"], name=f"waug{li}")
                wt = pp.tile([128, 136], f32, name=f"waug_sb{li}")
                nc.sync.dma_start(out=wt[:], in_=wh[:, :])
                waug_sb.append(wt)
                bh = nc.inline_tensor(consts[f"brep{li}"], name=f"brep{li}")
                bt = pp.tile([128, consts[f"brep{li}"].shape[1]], f32,
                             name=f"brep_sb{li}")
                nc.sync.dma_start(out=bt[:], in_=bh[:, :])
                brep_sb.append(bt)

            # dummy table row (feat=0, el_hi=-1e30, el_lo=0)
            dummy_h = nc.inline_tensor(consts["dummyrow"], name="dummyrow")

            # ---- DRAM scratch ----
            table = dp.tile([TROWS, TCOLS], bf16)
            er_tab = dp.tile([PAIR, ERCOLS], f32)
            ag_feat = dp.tile([OWN, TCOLS], bf16)
            ag_er = dp.tile([OWN, ERCOLS], f32)
            partial = dp.tile([PAIR, 132], f32)
            own_sum = dp.tile([OWN, 132], f32)

            nc.sync.dma_start(out=table[DUMMY:DUMMY + 1, :], in_=dummy_h[:, :])

            me = nc.my_core_id if hasattr(nc, "my_core_id") else None
            groups_pair = [[2 * k, 2 * k + 1] for k in range(4)]
            groups_quad = [[0, 2, 4, 6], [1, 3, 5, 7]]

            for li in range(3):
                src_hT = hT if li % 2 == 0 else hT2
                dst_hT = hT2 if li % 2 == 0 else hT
                last = li == 2

                # ---- projection of own rows + table assembly ----
                with tc.tile_pool(name=f"prj{li}", bufs=3) as sp, \
                     tc.tile_pool(name=f"prjps{li}", bufs=2, space="PSUM") as ps:
                    tabrow = sp.tile([128, OWNBLK, TCOLS], bf16, name=f"tabrow{li}",
                                     tag="tabrow", bufs=1)
                    errow = sp.tile([128, OWNBLK, ERCOLS], f32, name=f"errow{li}",
                                    tag="errow", bufs=1)
                    for t in range(OWNBLK):
                        pj = ps.tile([128, 136], f32, space="PSUM")
                        nc.tensor.matmul(pj[:], lhsT=src_hT[:, t * 128:(t + 1) * 128],
                                         rhs=waug_sb[li][:], start=True, stop=True)
                        # bf16 hi part: feat + el_hi
                        nc.vector.tensor_copy(tabrow[:, t, 0:132], pj[:, 0:132])
                        # el_lo = el - fp32(el_hi)
                        nc.vector.tensor_tensor(
                            out=tabrow[:, t, 132:136], in0=pj[:, 128:132],
                            in1=tabrow[:, t, 128:132], op=OP.subtract)
                        nc.scalar.activation(errow[:, t, 0:4], pj[:, 132:136], AF.Copy)
                    nc.sync.dma_start(
                        out=ag_feat[:, :].rearrange("(t p) c -> p t c", p=128),
                        in_=tabrow[:])
                    nc.sync.dma_start(
                        out=ag_er[:, :].rearrange("(t p) c -> p t c", p=128),
                        in_=errow[:])

                nc.gpsimd.collective_compute(
                    "AllGather", mybir.AluOpType.bypass,
                    replica_groups=groups_quad,
                    ins=[ag_feat[:, :]], outs=[table[0:HALF, :]])
                nc.gpsimd.collective_compute(
                    "AllGather", mybir.AluOpType.bypass,
                    replica_groups=groups_pair,
                    ins=[ag_er[:, :]], outs=[er_tab[:, :]])

                # ---- edge phase ----
                with tc.tile_pool(name=f"gt{li}", bufs=3) as gp, \
                     tc.tile_pool(name=f"ms{li}", bufs=4) as mp, \
                     tc.tile_pool(name=f"sm{li}", bufs=4) as smp, \
                     tc.tile_pool(name=f"ex{li}", bufs=4) as xp, \
                     tc.tile_pool(name=f"pb{li}", bufs=4) as pbp, \
                     tc.tile_pool(name=f"sg{li}", bufs=6, space="PSUM") as sgps:
                    seg = None
                    for (c0, clen) in chunks:
                        g = gp.tile([128, CHUNK, TCOLS], bf16, tag="g")
                        nc.gpsimd.dma_gather(
                            out_ap=g[:, 0:clen, :], in_ap=table[:, :],
                            idxs_ap=idx_sb[:, c0 * 8:(c0 + clen) * 8],
                            num_idxs=clen * 128, num_idxs_reg=clen * 128,
                            elem_size=TCOLS, single_packet=False)
                        ger = gp.tile([128, CHUNK, ERCOLS], f32, tag="ger")
                        nc.gpsimd.dma_gather(
                            out_ap=ger[:, 0:clen, :], in_ap=er_tab[:, :],
                            idxs_ap=idx2_sb[:, c0 * 8:(c0 + clen) * 8],
                            num_idxs=clen * 128, num_idxs_reg=clen * 128,
                            elem_size=ERCOLS, single_packet=False)
                        for g0 in range(0, clen, 4):
                            gl = min(4, clen - g0)
                            # masks: smat[p, t, i] = (slot[p, c0+g0+t] == i)
                            smat = smp.tile([128, 4, 128], bf16, tag="smat")
                            nc.vector.tensor_tensor(
                                out=smat[:, 0:gl, :],
                                in0=slot_sb[:, c0 + g0:c0 + g0 + gl, :]
                                    .to_broadcast([128, gl, 128]),
                                in1=iota_sb[:].rearrange("p (t i) -> p t i", t=1)
                                    .to_broadcast([128, gl, 128]),
                                op=OP.is_equal)
                            # e = el_hi + el_lo + er ; lrelu; exp
                            e4 = xp.tile([128, 4, 4], f32, tag="e4")
                            nc.vector.tensor_tensor(
                                out=e4[:, 0:gl, :], in0=g[:, g0:g0 + gl, 128:132],
                                in1=g[:, g0:g0 + gl, 132:136], op=OP.add)
                            nc.vector.tensor_tensor(
                                out=e4[:, 0:gl, :], in0=e4[:, 0:gl, :],
                                in1=ger[:, g0:g0 + gl, 0:4], op=OP.add)
                            t4 = xp.tile([128, 4, 4], f32, tag="t4")
                            nc.scalar.activation(t4[:, 0:gl, :], e4[:, 0:gl, :],
                                                 AF.Copy, scale=NEG)
                            nc.vector.tensor_tensor(
                                out=e4[:, 0:gl, :], in0=e4[:, 0:gl, :],
                                in1=t4[:, 0:gl, :], op=OP.max)
                            ex4 = xp.tile([128, 4, 4, 1], f32, tag="ex4")
                            nc.scalar.activation(ex4[:, 0:gl, :, 0], e4[:, 0:gl, :],
                                                 AF.Exp)
                            # messages
                            m4 = mp.tile([128, 4, 132], bf16, tag="m4")
                            nc.scalar.activation(m4[:, 0:gl, 128:132],
                                                 ex4[:, 0:gl, :, 0], AF.Copy)
                            nc.vector.tensor_tensor(
                                out=m4[:, 0:gl, 0:128], in0=g[:, g0:g0 + gl, 0:128],
                                in1=ex4[:, 0:gl, :, :].to_broadcast([128, gl, 4, 32]),
                                op=OP.mult)
                            for t in range(gl):
                                gt = c0 + g0 + t
                                b = int(tile_block[gt])
                                if tile_first[gt]:
                                    seg = sgps.tile([128, 132], f32, space="PSUM",
                                                    tag="seg", name=f"seg{li}_{b}")
                                nc.tensor.matmul(
                                    seg[:], lhsT=smat[:, t, :], rhs=m4[:, t, :],
                                    start=bool(tile_first[gt]),
                                    stop=bool(tile_last[gt]))
                                if tile_last[gt]:
                                    pb = pbp.tile([128, 132], f32, tag="pb")
                                    nc.scalar.activation(pb[:], seg[:], AF.Copy)
                                    nc.sync.dma_start(
                                        out=partial[b * 128:(b + 1) * 128, :],
                                        in_=pb[:])

                nc.gpsimd.collective_compute(
                    "ReduceScatter", mybir.AluOpType.add,
                    replica_groups=groups_pair,
                    ins=[partial[:, :]], outs=[own_sum[:, :]])

                # ---- post-processing of own rows ----
                with tc.tile_pool(name=f"po{li}", bufs=4) as pop, \
                     tc.tile_pool(name=f"pops{li}", bufs=2, space="PSUM") as tps:
                    osum = pop.tile([128, OWNBLK, 132], f32, name=f"osum{li}",
                                    tag="osum", bufs=1)
                    nc.sync.dma_start(
                        out=osum[:],
                        in_=own_sum[:, :].rearrange("(t p) c -> p t c", p=128))
                    for b in range(OWNBLK):
                        rows = 128 if b < OWNBLK - 1 else NPC - 128 * (OWNBLK - 1)
                        den = pop.tile([128, 4], f32, tag="den")
                        nc.vector.tensor_scalar_max(den[:], osum[:, b, 128:132], EPS)
                        rec = pop.tile([128, 4, 1], f32, tag="rec")
                        nc.vector.reciprocal(rec[:, :, 0], den[:])
                        o = pop.tile([128, 4, 32], f32, tag="o")
                        nc.vector.tensor_tensor(
                            out=o[:], in0=osum[:, b, 0:128],
                            in1=rec[:].to_broadcast([128, 4, 32]), op=OP.mult)
                        if not last:
                            o2 = pop.tile([128, 128], f32, tag="o2")
                            nc.vector.tensor_tensor(
                                out=o2[:], in0=o[:], in1=brep_sb[li][:], op=OP.add)
                            # ELU: max(x,0) + exp(min(x,0)) - 1
                            mn = pop.tile([128, 128], f32, tag="mn")
                            nc.vector.tensor_scalar_min(mn[:], o2[:], 0.0)
                            exn = pop.tile([128, 128], f32, tag="exn")
                            nc.scalar.activation(exn[:], mn[:], AF.Exp)
                            nc.vector.tensor_scalar_max(o2[:], o2[:], 0.0)
                            nc.vector.tensor_tensor(out=o2[:], in0=o2[:],
                                                    in1=exn[:], op=OP.add)
                            nc.vector.tensor_scalar_add(o2[:], o2[:], -1.0)
                            # transpose into next layer's hT
                            tp = tps.tile([128, 128], f32, space="PSUM")
                            nc.tensor.matmul(tp[:], lhsT=o2[:], rhs=ident_sb[:],
                                             start=True, stop=True)
                            nc.scalar.activation(dst_hT[:, b * 128:(b + 1) * 128],
                                                 tp[:], AF.Copy)
                        else:
                            r1 = pop.tile([128, 32], f32, tag="r1")
                            nc.vector.tensor_tensor(out=r1[:], in0=o[:, 0, :],
                                                    in1=o[:, 1, :], op=OP.add)
                            r2 = pop.tile([128, 32], f32, tag="r2")
                            nc.vector.tensor_tensor(out=r2[:], in0=o[:, 2, :],
                                                    in1=o[:, 3, :], op=OP.add)
                            nc.vector.tensor_tensor(out=r1[:], in0=r1[:],
                                                    in1=r2[:], op=OP.add)
                            nc.vector.tensor_scalar_mul(r1[:], r1[:], 0.25)
                            nc.vector.tensor_tensor(out=r1[:], in0=r1[:],
                                                    in1=brep_sb[li][:], op=OP.add)
                            nc.sync.dma_start(
                                out=y_out[b * 128:b * 128 + rows, :],
                                in_=r1[0:rows, :])
    nc.finalize()
    return nc


def kernel(x, src, dst, W0, al0, ar0, b0, W1, al1, ar1, b1, W2, al2, ar2, b2):
    from concourse.bass_utils import run_bass_kernel_spmd

    x = np.asarray(x, dtype=np.float32)
    key = (hash(np.asarray(src).tobytes()) ^ hash(np.asarray(dst).tobytes()))
    if "pre" not in _cache or _cache.get("prekey") != key:
        _cache["pre"] = _preprocess(src, dst)
        _cache["prekey"] = key
    cores, tile_block, tile_first, tile_last, T = _cache["pre"]

    consts = {}
    for li, (W, al, ar, b) in enumerate(
            [(W0, al0, ar0, b0), (W1, al1, ar1, b1), (W2, al2, ar2, b2)]):
        consts[f"Waug{li}"] = _augment(np.asarray(W, np.float32),
                                       np.asarray(al, np.float32),
                                       np.asarray(ar, np.float32))
        b = np.asarray(b, np.float32)
        if li < 2:
            consts[f"brep{li}"] = np.tile(b.reshape(1, 128), (128, 1))
        else:
            consts[f"brep{li}"] = np.tile(b.reshape(H, D).mean(0).reshape(1, D),
                                          (128, 1))
    dummy = np.zeros((1, TCOLS), ml_dtypes.bfloat16)
    dummy[0, 128:132] = ml_dtypes.bfloat16(-1e30)
    consts["dummyrow"] = dummy

    ck = key ^ hash(consts["Waug0"].tobytes())
    if "nc" not in _cache or _cache.get("nckey") != ck:
        _cache["nc"] = _build(tile_block, tile_first, tile_last, T, consts)
        _cache["nckey"] = ck
    nc = _cache["nc"]

    in_maps = []
    for c in range(NCORE):
        lo, hi = _own_rows(c)
        xT = np.zeros((128, OWN), np.float32)
        xT[:, 0:NPC] = x[lo:hi].T
        in_maps.append(dict(xT=xT, idxw=cores[c]["idxw"],
                            idx2w=cores[c]["idx2w"],
                            slot=np.asarray(cores[c]["slot"])))
    r = run_bass_kernel_spmd(nc, in_maps, list(range(NCORE)))
    y = np.zeros((N, D), np.float32)
    for c in range(NCORE):
        lo, hi = _own_rows(c)
        y[lo:hi] = r.results[c]["y"]
    return y


# revision 9
# speedup vs baseline: 1.4097x; 1.2470x over previous
"""3-layer GAT on 8 Trainium2 NeuronCores (Bass/Tile).

Sharding: 2D graph partition. Pair q = cores {2q, 2q+1} aggregates the dst
nodes of strips [q*6250,(q+1)*6250) and [25000+q*6250, 25000+(q+1)*6250);
even cores take edges with src < 25000, odd cores the rest. Node ownership:
core 2k owns rows [k*6250,(k+1)*6250), core 2k+1 owns [25000+k*6250, ...).

Per layer: each core projects its own rows (feat|el|er via an augmented
weight matrix), AllGathers a bf16 gather-table (feat_hi|el_hi|el_lo) across
its src-half quad and an fp32 er-table across its pair, then streams edges
(pre-sorted by dst, padded per 128-dst block) through: dma_gather of source
rows, edge softmax without segment-max (e-values are small), messages
accumulated per dst block by one-hot-mask matmuls into PSUM. Partial sums
are pairwise ReduceScattered, then divide/bias/ELU (head-mean on the last
layer) produce the output rows each core owns.
"""

import numpy as np
import ml_dtypes

N = 50000
E = 800000
F = 128                  # input feats and hidden width (4 heads x 32)
H = 4
D = 32
NEG = 0.2
NCORE = 8
NPC = 6250               # nodes owned per core
OWN = 6272               # 49*128, padded own rows
OWNBLK = 49
PAIR = 12544             # 98*128 dst slots per pair
NBLK = 98
HALF = 25088             # 4*OWN rows per src-half table
TROWS = 25216            # HALF + 128 (dummy row at HALF)
DUMMY = HALF
TCOLS = 256              # bf16 cols: feat_hi(128) | el_hi(4) | el_lo(4) | pad
ERCOLS = 64              # fp32 cols: er(4) | pad
CHUNK = 32               # tiles per dma_gather call
GROUP = 8                # tiles per vector-op batch
EPS = 1e-30

_cache = {}


def _preprocess(src, dst):
    src = np.asarray(src).astype(np.int64)
    dst = np.asarray(dst).astype(np.int64)
    q = np.where(dst < 25000, dst // NPC, (dst - 25000) // NPC)
    s = (src >= 25000).astype(np.int64)
    core_of = 2 * q + s
    # pair-local dst row in [0, 12544): strip A -> [0,6250), strip B -> 6272+
    rloc = np.where(dst < 25000, dst - q * NPC, OWN + (dst - 25000 - q * NPC))
    blk = rloc // 128
    slot = rloc % 128
    # table-local src index within its half
    ks = np.where(src < 25000, src // NPC, (src - 25000) // NPC)
    tloc = np.where(src < 25000, OWN * ks + src - ks * NPC,
                    OWN * ks + (src - 25000) - ks * NPC)

    # per (core, block) edge lists; uniform tiles per block across cores
    counts = np.zeros((NCORE, NBLK), np.int64)
    for c in range(NCORE):
        m = core_of == c
        counts[c] = np.bincount(blk[m], minlength=NBLK)
    T_b = np.maximum(1, np.ceil(counts.max(axis=0) / 128).astype(np.int64))
    T = int(T_b.sum())
    Tpad = ((T + 7) // 8) * 8          # pad tile count to a multiple of GROUP
    extra = Tpad - T
    T_b[-1] += extra
    T = Tpad

    per_core = []
    order = np.lexsort((rloc, core_of))
    srt_core = core_of[order]
    srt_rloc = rloc[order]
    srt_tloc = tloc[order]
    srt_slot = slot[order]
    srt_blk = srt_rloc // 128
    for c in range(NCORE):
        sel = srt_core == c
        cb = srt_blk[sel]
        ct = srt_tloc[sel]
        cr = srt_rloc[sel]
        csl = srt_slot[sel]
        idx_t = np.full(T * 128, DUMMY, np.int64)      # feat-table row per edge
        idx_e = np.zeros(T * 128, np.int64)            # er-table row per edge
        slots = np.zeros(T * 128, np.int64)
        pos = 0
        start = np.searchsorted(cb, np.arange(NBLK))
        end = np.searchsorted(cb, np.arange(NBLK) + 1)
        for b in range(NBLK):
            n = end[b] - start[b]
            idx_t[pos:pos + n] = ct[start[b]:end[b]]
            idx_e[pos:pos + n] = cr[start[b]:end[b]]
            slots[pos:pos + n] = csl[start[b]:end[b]]
            pos += int(T_b[b]) * 128
        per_core.append((idx_t, idx_e, slots))

    # tile -> block map
    tile_block = np.repeat(np.arange(NBLK), T_b)
    tile_first = np.zeros(T, bool)
    tile_last = np.zeros(T, bool)
    p = 0
    for b in range(NBLK):
        tile_first[p] = True
        tile_last[p + int(T_b[b]) - 1] = True
        p += int(T_b[b])

    def wrap16(a):
        # value i of each 128-group at [i%16, i//16], replicated per 16 rows
        t = a.reshape(-1, 128)                     # [T, 128]
        w = t.reshape(t.shape[0], 8, 16)           # [T, 8, 16]
        w = w.transpose(2, 0, 1).reshape(16, -1)   # [16, T*8]
        return np.tile(w, (8, 1)).astype(np.int16) # [128, T*8]

    cores = []
    for c in range(NCORE):
        idx_t, idx_e, slots = per_core[c]
        cores.append(dict(
            idxw=wrap16(idx_t),
            idx2w=wrap16(idx_e),
            slot=slots.reshape(T, 128).T.astype(ml_dtypes.bfloat16).copy(),
        ))
    return cores, tile_block, tile_first, tile_last, T


def _own_rows(c):
    k = c // 2
    if c % 2 == 0:
        return k * NPC, (k + 1) * NPC
    return 25000 + k * NPC, 25000 + (k + 1) * NPC


def _augment(W, al, ar):
    dout = W.shape[1] // H
    Wal = np.stack([W[:, h * dout:(h + 1) * dout] @ al[h] for h in range(H)], 1)
    War = np.stack([W[:, h * dout:(h + 1) * dout] @ ar[h] for h in range(H)], 1)
    return np.concatenate([W, Wal, War], 1).astype(np.float32)  # [128, 136]


def _build(tile_block, tile_first, tile_last, T, consts, no_cc=False):
    import concourse.bass as bass
    import concourse.bacc as bacc
    import concourse.tile as tile
    from concourse import mybir
    from concourse.library_config import mlp

    f32 = mybir.dt.float32
    bf16 = mybir.dt.bfloat16
    i16 = mybir.dt.int16
    AF = mybir.ActivationFunctionType
    OP = mybir.AluOpType

    nc = bacc.Bacc(num_devices=NCORE)
    xT_in = nc.declare_dram_parameter("xT", [128, OWN], f32, isOutput=False)
    idxw_in = nc.declare_dram_parameter("idxw", [128, T * 8], i16, isOutput=False)
    idx2w_in = nc.declare_dram_parameter("idx2w", [128, T * 8], i16, isOutput=False)
    slot_in = nc.declare_dram_parameter("slot", [128, T], bf16, isOutput=False)
    y_out = nc.declare_dram_parameter("y", [NPC, D], f32, isOutput=True)

    chunks = []
    t0 = 0
    while t0 < T:
        chunks.append((t0, min(CHUNK, T - t0)))
        t0 += CHUNK

    with tile.TileContext(nc) as tc:
        with tc.tile_pool(name="persist", bufs=1) as pp, \
             tc.tile_pool(name="dram", bufs=1, space="DRAM") as dp:
            nc.gpsimd.load_library(mlp)

            # ---- persistent SBUF state ----
            idx_sb = pp.tile([128, T * 8], i16)
            nc.sync.dma_start(out=idx_sb[:], in_=idxw_in[:, :])
            idx2_sb = pp.tile([128, T * 8], i16)
            nc.sync.dma_start(out=idx2_sb[:], in_=idx2w_in[:, :])
            slot_sb = pp.tile([128, T, 1], bf16)
            nc.sync.dma_start(out=slot_sb[:, :, 0], in_=slot_in[:, :])
            hT = pp.tile([128, OWN], f32)
            nc.sync.dma_start(out=hT[:], in_=xT_in[:, :])
            hT2 = pp.tile([128, OWN], f32)

            iota_h = nc.inline_tensor(
                np.tile(np.arange(128).astype(ml_dtypes.bfloat16), (128, 1)),
                name="iota_row")
            iota_sb = pp.tile([128, 128], bf16)
            nc.sync.dma_start(out=iota_sb[:], in_=iota_h[:, :])
            ident_h = nc.inline_tensor(np.eye(128, dtype=np.float32), name="ident")
            ident_sb = pp.tile([128, 128], f32)
            nc.sync.dma_start(out=ident_sb[:], in_=ident_h[:, :])

            waug_sb = []
            brep_sb = []
            for li in range(3):
                wh = nc.inline_tensor(consts[f"Waug{li}"], name=f"waug{li}")
                wt = pp.tile([128, 136], f32, name=f"waug_sb{li}")
                nc.sync.dma_start(out=wt[:], in_=wh[:, :])
                waug_sb.append(wt)
                bh = nc.inline_tensor(consts[f"brep{li}"], name=f"brep{li}")
                bt = pp.tile([128, consts[f"brep{li}"].shape[1]], f32,
                             name=f"brep_sb{li}")
                nc.sync.dma_start(out=bt[:], in_=bh[:, :])
                brep_sb.append(bt)

            # dummy table row (feat=0, el_hi=-1e30, el_lo=0)
            dummy_h = nc.inline_tensor(consts["dummyrow"], name="dummyrow")

            # ---- DRAM scratch ----
            table = dp.tile([TROWS, TCOLS], bf16)
            er_tab = dp.tile([PAIR, ERCOLS], f32)
            ag_feat = dp.tile([OWN, TCOLS], bf16)
            ag_er = dp.tile([OWN, ERCOLS], f32)
            partial = dp.tile([PAIR, 132], f32)
            own_sum = dp.tile([OWN, 132], f32)

            nc.sync.dma_start(out=table[DUMMY:DUMMY + 1, :], in_=dummy_h[:, :])

            me = nc.my_core_id if hasattr(nc, "my_core_id") else None
            groups_pair = [[2 * k, 2 * k + 1] for k in range(4)]
            groups_quad = [[0, 2, 4, 6], [1, 3, 5, 7]]

            for li in range(3):
                src_hT = hT if li % 2 == 0 else hT2
                dst_hT = hT2 if li % 2 == 0 else hT
                last = li == 2

                # ---- projection of own rows + table assembly ----
                with tc.tile_pool(name=f"prj{li}", bufs=3) as sp, \
                     tc.tile_pool(name=f"prjps{li}", bufs=2, space="PSUM") as ps:
                    tabrow = sp.tile([128, OWNBLK, TCOLS], bf16, name=f"tabrow{li}",
                                     tag="tabrow", bufs=1)
                    errow = sp.tile([128, OWNBLK, ERCOLS], f32, name=f"errow{li}",
                                    tag="errow", bufs=1)
                    for t in range(OWNBLK):
                        pj = ps.tile([128, 136], f32, space="PSUM")
                        nc.tensor.matmul(pj[:], lhsT=src_hT[:, t * 128:(t + 1) * 128],
                                         rhs=waug_sb[li][:], start=True, stop=True)
                        # bf16 hi part: feat + el_hi
                        nc.vector.tensor_copy(tabrow[:, t, 0:132], pj[:, 0:132])
                        # el_lo = el - fp32(el_hi)
                        nc.vector.tensor_tensor(
                            out=tabrow[:, t, 132:136], in0=pj[:, 128:132],
                            in1=tabrow[:, t, 128:132], op=OP.subtract)
                        nc.scalar.activation(errow[:, t, 0:4], pj[:, 132:136], AF.Copy)
                    nc.sync.dma_start(
                        out=ag_feat[:, :].rearrange("(t p) c -> p t c", p=128),
                        in_=tabrow[:])
                    nc.sync.dma_start(
                        out=ag_er[:, :].rearrange("(t p) c -> p t c", p=128),
                        in_=errow[:])

                if no_cc:
                    for rep in range(4):
                        nc.sync.dma_start(out=table[rep * OWN:(rep + 1) * OWN, :],
                                          in_=ag_feat[:, :])
                    for rep in range(2):
                        nc.sync.dma_start(out=er_tab[rep * OWN:(rep + 1) * OWN, :],
                                          in_=ag_er[:, :])
                else:
                    nc.gpsimd.collective_compute(
                        "AllGather", mybir.AluOpType.bypass,
                        replica_groups=groups_quad,
                        ins=[ag_feat[:, :]], outs=[table[0:HALF, :]])
                    nc.gpsimd.collective_compute(
                        "AllGather", mybir.AluOpType.bypass,
                        replica_groups=groups_pair,
                        ins=[ag_er[:, :]], outs=[er_tab[:, :]])

                # ---- edge phase ----
                with tc.tile_pool(name=f"gt{li}", bufs=2) as gp, \
                     tc.tile_pool(name=f"ms{li}", bufs=4) as mp, \
                     tc.tile_pool(name=f"sm{li}", bufs=4) as smp, \
                     tc.tile_pool(name=f"ex{li}", bufs=4) as xp, \
                     tc.tile_pool(name=f"pb{li}", bufs=4) as pbp, \
                     tc.tile_pool(name=f"sg{li}", bufs=6, space="PSUM") as sgps:
                    seg = None
                    for (c0, clen) in chunks:
                        g = gp.tile([128, CHUNK, TCOLS], bf16, tag="g")
                        nc.gpsimd.dma_gather(
                            out_ap=g[:, 0:clen, :], in_ap=table[:, :],
                            idxs_ap=idx_sb[:, c0 * 8:(c0 + clen) * 8],
                            num_idxs=clen * 128, num_idxs_reg=clen * 128,
                            elem_size=TCOLS, single_packet=False)
                        ger = gp.tile([128, CHUNK, ERCOLS], f32, tag="ger")
                        nc.gpsimd.dma_gather(
                            out_ap=ger[:, 0:clen, :], in_ap=er_tab[:, :],
                            idxs_ap=idx2_sb[:, c0 * 8:(c0 + clen) * 8],
                            num_idxs=clen * 128, num_idxs_reg=clen * 128,
                            elem_size=ERCOLS, single_packet=False)
                        for g0 in range(0, clen, GROUP):
                            gl = min(GROUP, clen - g0)
                            # masks: smat[p, t, i] = (slot[p, c0+g0+t] == i)
                            smat = smp.tile([128, GROUP, 128], bf16, tag="smat")
                            nc.vector.tensor_tensor(
                                out=smat[:, 0:gl, :],
                                in0=slot_sb[:, c0 + g0:c0 + g0 + gl, :]
                                    .to_broadcast([128, gl, 128]),
                                in1=iota_sb[:].rearrange("p (t i) -> p t i", t=1)
                                    .to_broadcast([128, gl, 128]),
                                op=OP.is_equal)
                            # e = el_hi + el_lo + er ; lrelu; exp
                            e4 = xp.tile([128, GROUP, 4], f32, tag="e4")
                            nc.vector.tensor_tensor(
                                out=e4[:, 0:gl, :], in0=g[:, g0:g0 + gl, 128:132],
                                in1=g[:, g0:g0 + gl, 132:136], op=OP.add)
                            nc.vector.tensor_tensor(
                                out=e4[:, 0:gl, :], in0=e4[:, 0:gl, :],
                                in1=ger[:, g0:g0 + gl, 0:4], op=OP.add)
                            t4 = xp.tile([128, GROUP, 4], f32, tag="t4")
                            nc.scalar.activation(t4[:, 0:gl, :], e4[:, 0:gl, :],
                                                 AF.Copy, scale=NEG)
                            nc.vector.tensor_tensor(
                                out=e4[:, 0:gl, :], in0=e4[:, 0:gl, :],
                                in1=t4[:, 0:gl, :], op=OP.max)
                            ex4 = xp.tile([128, GROUP, 4, 1], f32, tag="ex4")
                            nc.scalar.activation(ex4[:, 0:gl, :, 0], e4[:, 0:gl, :],
                                                 AF.Exp)
                            # messages
                            m4 = mp.tile([128, GROUP, 132], bf16, tag="m4")
                            nc.scalar.activation(m4[:, 0:gl, 128:132],
                                                 ex4[:, 0:gl, :, 0], AF.Copy)
                            nc.vector.tensor_tensor(
                                out=m4[:, 0:gl, 0:128], in0=g[:, g0:g0 + gl, 0:128],
                                in1=ex4[:, 0:gl, :, :].to_broadcast([128, gl, 4, 32]),
                                op=OP.mult)
                            for t in range(gl):
                                gt = c0 + g0 + t
                                b = int(tile_block[gt])
                                if tile_first[gt]:
                                    seg = sgps.tile([128, 132], f32, space="PSUM",
                                                    tag="seg", name=f"seg{li}_{b}")
                                nc.tensor.matmul(
                                    seg[:], lhsT=smat[:, t, :], rhs=m4[:, t, :],
                                    start=bool(tile_first[gt]),
                                    stop=bool(tile_last[gt]))
                                if tile_last[gt]:
                                    pb = pbp.tile([128, 132], f32, tag="pb")
                                    nc.scalar.activation(pb[:], seg[:], AF.Copy)
                                    nc.sync.dma_start(
                                        out=partial[b * 128:(b + 1) * 128, :],
                                        in_=pb[:])

                if no_cc:
                    nc.sync.dma_start(out=own_sum[:, :], in_=partial[0:OWN, :])
                else:
                    nc.gpsimd.collective_compute(
                        "ReduceScatter", mybir.AluOpType.add,
                        replica_groups=groups_pair,
                        ins=[partial[:, :]], outs=[own_sum[:, :]])

                # ---- post-processing of own rows ----
                with tc.tile_pool(name=f"po{li}", bufs=4) as pop, \
                     tc.tile_pool(name=f"pops{li}", bufs=2, space="PSUM") as tps:
                    osum = pop.tile([128, OWNBLK, 132], f32, name=f"osum{li}",
                                    tag="osum", bufs=1)
                    nc.sync.dma_start(
                        out=osum[:],
                        in_=own_sum[:, :].rearrange("(t p) c -> p t c", p=128))
                    for b in range(OWNBLK):
                        rows = 128 if b < OWNBLK - 1 else NPC - 128 * (OWNBLK - 1)
                        den = pop.tile([128, 4], f32, tag="den")
                        nc.vector.tensor_scalar_max(den[:], osum[:, b, 128:132], EPS)
                        rec = pop.tile([128, 4, 1], f32, tag="rec")
                        nc.vector.reciprocal(rec[:, :, 0], den[:])
                        o = pop.tile([128, 4, 32], f32, tag="o")
                        nc.vector.tensor_tensor(
                            out=o[:], in0=osum[:, b, 0:128],
                            in1=rec[:].to_broadcast([128, 4, 32]), op=OP.mult)
                        if not last:
                            o2 = pop.tile([128, 128], f32, tag="o2")
                            nc.vector.tensor_tensor(
                                out=o2[:], in0=o[:], in1=brep_sb[li][:], op=OP.add)
                            # ELU: max(x,0) + exp(min(x,0)) - 1
                            mn = pop.tile([128, 128], f32, tag="mn")
                            nc.vector.tensor_scalar_min(mn[:], o2[:], 0.0)
                            exn = pop.tile([128, 128], f32, tag="exn")
                            nc.scalar.activation(exn[:], mn[:], AF.Exp)
                            nc.vector.tensor_scalar_max(o2[:], o2[:], 0.0)
                            nc.vector.tensor_tensor(out=o2[:], in0=o2[:],
                                                    in1=exn[:], op=OP.add)
                            nc.vector.tensor_scalar_add(o2[:], o2[:], -1.0)
                            # transpose into next layer's hT
                            tp = tps.tile([128, 128], f32, space="PSUM")
                            nc.tensor.matmul(tp[:], lhsT=o2[:], rhs=ident_sb[:],
                                             start=True, stop=True)
                            nc.scalar.activation(dst_hT[:, b * 128:(b + 1) * 128],
                                                 tp[:], AF.Copy)
                        else:
                            r1 = pop.tile([128, 32], f32, tag="r1")
                            nc.vector.tensor_tensor(out=r1[:], in0=o[:, 0, :],
                                                    in1=o[:, 1, :], op=OP.add)
                            r2 = pop.tile([128, 32], f32, tag="r2")
                            nc.vector.tensor_tensor(out=r2[:], in0=o[:, 2, :],
                                                    in1=o[:, 3, :], op=OP.add)
                            nc.vector.tensor_tensor(out=r1[:], in0=r1[:],
                                                    in1=r2[:], op=OP.add)
                            nc.vector.tensor_scalar_mul(r1[:], r1[:], 0.25)
                            nc.vector.tensor_tensor(out=r1[:], in0=r1[:],
                                                    in1=brep_sb[li][:], op=OP.add)
                            nc.sync.dma_start(
                                out=y_out[b * 128:b * 128 + rows, :],
                                in_=r1[0:rows, :])
    nc.finalize()
    return nc


def kernel(x, src, dst, W0, al0, ar0, b0, W1, al1, ar1, b1, W2, al2, ar2, b2):
    from concourse.bass_utils import run_bass_kernel_spmd

    x = np.asarray(x, dtype=np.float32)
    key = (hash(np.asarray(src).tobytes()) ^ hash(np.asarray(dst).tobytes()))
    if "pre" not in _cache or _cache.get("prekey") != key:
        _cache["pre"] = _preprocess(src, dst)
        _cache["prekey"] = key
    cores, tile_block, tile_first, tile_last, T = _cache["pre"]

    consts = {}
    for li, (W, al, ar, b) in enumerate(
            [(W0, al0, ar0, b0), (W1, al1, ar1, b1), (W2, al2, ar2, b2)]):
        consts[f"Waug{li}"] = _augment(np.asarray(W, np.float32),
                                       np.asarray(al, np.float32),
                                       np.asarray(ar, np.float32))
        b = np.asarray(b, np.float32)
        if li < 2:
            consts[f"brep{li}"] = np.tile(b.reshape(1, 128), (128, 1))
        else:
            consts[f"brep{li}"] = np.tile(b.reshape(H, D).mean(0).reshape(1, D),
                                          (128, 1))
    dummy = np.zeros((1, TCOLS), ml_dtypes.bfloat16)
    dummy[0, 128:132] = ml_dtypes.bfloat16(-1e30)
    consts["dummyrow"] = dummy

    ck = key ^ hash(consts["Waug0"].tobytes())
    if "nc" not in _cache or _cache.get("nckey") != ck:
        _cache["nc"] = _build(tile_block, tile_first, tile_last, T, consts)
        _cache["nckey"] = ck
    nc = _cache["nc"]

    in_maps = []
    for c in range(NCORE):
        lo, hi = _own_rows(c)
        xT = np.zeros((128, OWN), np.float32)
        xT[:, 0:NPC] = x[lo:hi].T
        in_maps.append(dict(xT=xT, idxw=cores[c]["idxw"],
                            idx2w=cores[c]["idx2w"],
                            slot=np.asarray(cores[c]["slot"])))
    r = run_bass_kernel_spmd(nc, in_maps, list(range(NCORE)))
    y = np.zeros((N, D), np.float32)
    for c in range(NCORE):
        lo, hi = _own_rows(c)
        y[lo:hi] = r.results[c]["y"]
    return y
